# revision 1
# baseline (speedup 1.0000x reference)
"""Trainium2 Bass kernel for a 2-layer ReLU RNN (batch_first) + linear head.

Problem shapes: B=256, T=512, I=512, H=1024, O=256 (fp32).
Sharding: data-parallel over batch across 8 NeuronCores (32 rows each);
weights replicated. No collectives.

Per-core design (all matmul operands bf16, fp32 PSUM accumulate):

  L0 step t: s0 = x_t @ W_ih0.T + h0 @ W_hh0.T   (12 k-tiles, fused)
  L1 step t: s1 = pre1_t + h1 @ W_hh1.T          (8 k-tiles in-step)
  pre1 = h0 @ W_ih1.T is computed in BATCHED 16-step chunks (full-M
  stream-bound GEMMs at ~852 ns/step-equivalent instead of LDW-bound
  ~1091 ns in-step); h0T lives in a 16-slot SBUF ring and L1 lags L0
  by 16 steps. The chunk GEMM's PSUM->SBUF copy folds bias1 (per-
  partition bias in j-layout) and writes pre1T directly in hT layout.

  In-step s-matmuls run 4x column-tiled (tile_size 128x32): col-tile v
  computes the interleaved output column set {j : (j//32)%4 == v}. This
  makes the PSUM layout ps[32v+b, 32w+j'] = s[b, 128w+32v+j'], which is
  32x32-block-transpose compatible: ONE DVE StreamTranspose of
  [128, 256] yields hT[jj, 32kt+b], so the PE does no transposes.
  L0 chain: DVE add(+interleaved bias0) -> DVE transpose -> ACT relu.
  L1 chain: DVE transpose (psum) -> DVE add(+pre1T) -> ACT relu.
  Chains execute under the other layer's matmul window.

  Per-step PE budget: L0 12 rounds + L1 8 rounds (LDW-bound ~136 ns
  each) + 852 ns batched projection ~= 3.6 us/step. The 4-way col
  tiling is the exact LDW/stream balance point (tiles^2 = H*1.2GHz /
  (32 * 2.4GHz) = 16); one LDW cannot feed multiple col groups
  (verified), FWL needs 128-col loads, DoubleRow fp8 is col-tiling-
  incompatible and accuracy-breaking, so this is the toolchain floor.

  Measured: 1.88 ms HW exec (5.70 ms phase-split f32r baseline;
  2.00 ms without the batched projection), rel err 4.6e-3. Both state
  chains are emitted in 128-col halves so the first half's relu lands
  within the other layer's matmul window -- monolithic chains stall the
  PE ~300 ns/step (measured 2.13 ms with a 13.6 us monolithic chunk).
  The projection is spread ONE jb-block per step over 8 steps (L1 lag
  CH+8) so each step's engine FIFO carries at most one phase copy --
  coarser 2-per-step spreading measured +50 us of FIFO-coupling stalls.
  Residual PE idle is ~77 us, of which ~50 us is pipeline fill/drain
  at the L0/L1 lag boundaries (one ~1 us chain-latency stall per solo
  step); steady state is ~98.5% PE-busy.

kernel(**inputs) takes the FULL unsharded inputs (keys as in the
reference setup_inputs) and returns the FULL [256, 256] output.
"""

import ml_dtypes
import numpy as np

import concourse.bass as bass
import concourse.tile as tile
import concourse.mybir as mybir
from concourse import bacc
from concourse.bass_utils import run_bass_kernel_spmd

F32 = mybir.dt.float32
BF16 = mybir.dt.bfloat16

B_FULL, T_FULL, I_DIM, H, O = 256, 512, 512, 1024, 256
N_CORES = 8
BL = B_FULL // N_CORES  # 32 batch rows per core
KX = I_DIM // 128       # 4 k-tiles of the input dim
KH = H // 128           # 8 k-tiles of the hidden dim
CH = 16                 # phase-C chunk length (steps); L1 lag


def _emit_step_mms(nc, ps, stat_tiles, w_sb, kb0, n_k):
    """One recurrence step's s-matmuls, 4x column-tiled.

    ps [128,256] f32 psum; stat_tiles: n_k stationary APs [128,32] bf16;
    w_sb k-block (kb0+i) columns [(kb0+i)*1024 + j] hold W[j, 128*i + r].
    Col-tile v streams columns {j : (j//32)%4 == v} via a strided AP.
    """
    for i in range(n_k):
        blk = w_sb[:, (kb0 + i) * 1024 : (kb0 + i + 1) * 1024].rearrange(
            "p (w f j) -> p w f j", f=4, j=32
        )
        for v in range(4):
            nc.tensor.matmul(
                ps[32 * v : 32 * v + 32, :],
                stat_tiles[i],
                blk[:, :, v : v + 1, :],
                start=(i == 0),
                stop=(i == n_k - 1),
                tile_position=(0, 32 * v),
                skip_group_check=True,
            )


def build_rnn(T):
    assert T % CH == 0
    nc = bacc.Bacc("TRN2", target_bir_lowering=False, debug=False)

    xTb_d = nc.dram_tensor("xTb", [I_DIM, T * BL], BF16, kind="ExternalInput").ap()
    w0_d = nc.dram_tensor("w0cat", [128, (KX + KH) * H], BF16, kind="ExternalInput").ap()
    w1_d = nc.dram_tensor("w1cat", [128, (KH + KH) * H], BF16, kind="ExternalInput").ap()
    fcw_d = nc.dram_tensor("fcwT", [128, KH * O], BF16, kind="ExternalInput").ap()
    b0_d = nc.dram_tensor("bias0il", [128, 256], F32, kind="ExternalInput").ap()
    b1_d = nc.dram_tensor("bias1pp", [128, KH], F32, kind="ExternalInput").ap()
    fcb_d = nc.dram_tensor("fcb", [BL, O], F32, kind="ExternalInput").ap()
    out_d = nc.dram_tensor("out", [BL, O], F32, kind="ExternalOutput").ap()

    PREF = 4  # xt DMA prefetch depth (steps ahead)

    with tile.TileContext(nc) as tc:
        with (
            tc.tile_pool(name="wpool", bufs=1) as wpool,
            tc.tile_pool(name="cpool", bufs=1) as cpool,
            tc.tile_pool(name="xt", bufs=PREF + 2) as xt_pool,
            tc.tile_pool(name="hT1", bufs=3) as hT1_pool,
            tc.tile_pool(name="sb0", bufs=2) as sb0_pool,
            tc.tile_pool(name="tr1", bufs=2) as tr1_pool,
            tc.tile_pool(name="ad1", bufs=2) as ad1_pool,
            tc.tile_pool(name="ps0", bufs=2, space="PSUM") as ps0_pool,
            tc.tile_pool(name="ps1", bufs=2, space="PSUM") as ps1_pool,
            tc.tile_pool(name="psc", bufs=2, space="PSUM") as psc_pool,
            tc.tile_pool(name="psh", bufs=1, space="PSUM") as psh_pool,
            tc.tile_pool(name="eout", bufs=1) as eo_pool,
        ):
            w0_sb = wpool.tile([128, (KX + KH) * H], BF16)
            w1_sb = wpool.tile([128, (KH + KH) * H], BF16)
            fcw_sb = wpool.tile([128, KH * O], BF16)
            b0_sb = cpool.tile([128, 256], F32)
            b1_sb = cpool.tile([128, KH], F32)
            fcb_sb = cpool.tile([BL, O], F32)
            # h0T ring: 2*CH slots of [128, 256]; slot u%(2CH) = step u's h0T
            ring = cpool.tile([128, 2 * CH * 256], BF16)
            # pre1T double ring: 2 chunks x 16 slots of [128, 256]
            ring2 = cpool.tile([128, 2 * CH * 256], BF16)
            # split big weight DMAs so step 0 only gates on the x-proj
            # blocks and queues stay parallel
            nc.sync.dma_start(w0_sb[:, : KX * H], w0_d[:, : KX * H])
            nc.sync.dma_start(w0_sb[:, KX * H : (KX + 4) * H],
                              w0_d[:, KX * H : (KX + 4) * H])
            nc.sync.dma_start(w0_sb[:, (KX + 4) * H :], w0_d[:, (KX + 4) * H :])
            nc.sync.dma_start(w1_sb[:, : KH * H], w1_d[:, : KH * H])
            nc.sync.dma_start(w1_sb[:, KH * H :], w1_d[:, KH * H :])
            nc.sync.dma_start(fcw_sb[:], fcw_d)
            nc.sync.dma_start(b0_sb[:], b0_d)
            nc.sync.dma_start(b1_sb[:], b1_d)
            nc.sync.dma_start(fcb_sb[:], fcb_d)

            xT_view = xTb_d.rearrange("(ki p) n -> p ki n", p=128)

            def emit_xt_dma(t):
                xt = xt_pool.tile([128, KX * BL], BF16, tag="xt")
                nc.sync.dma_start(
                    xt[:, :].rearrange("p (ki b) -> p ki b", ki=KX),
                    xT_view[:, :, t * BL : (t + 1) * BL],
                )
                return xt

            xt_tiles = {}
            for t in range(min(T, PREF)):
                xt_tiles[t] = emit_xt_dma(t)

            def l0_step(u):
                if u + PREF < T:
                    xt_tiles[u + PREF] = emit_xt_dma(u + PREF)
                xt = xt_tiles.pop(u)
                stats = [xt[:, 32 * k : 32 * k + 32] for k in range(KX)]
                n_k = KX
                if u > 0:
                    s_prev = 256 * ((u - 1) % (2 * CH))
                    prev = ring[:, s_prev : s_prev + 256]
                    stats += [prev[:, 32 * k : 32 * k + 32] for k in range(KH)]
                    n_k += KH
                ps0 = ps0_pool.tile([128, 256], F32, tag="ps0")
                _emit_step_mms(nc, ps0, stats, w0_sb, 0, n_k)
                # chain in 128-col halves (pipelined: half A's relu lands
                # before the next step's first hT matmul needs it):
                # +bias0 (interleaved) -> 32x32 transpose -> relu
                tr = ring[:, 256 * (u % (2 * CH)) : 256 * (u % (2 * CH)) + 256]
                sb = sb0_pool.tile([128, 256], BF16, tag="sb0")
                trt = sb0_pool.tile([128, 256], BF16, tag="tr0")
                for h in (0, 1):
                    cs = slice(128 * h, 128 * h + 128)
                    nc.vector.tensor_add(sb[:, cs], ps0[:, cs], b0_sb[:, cs])
                    nc.vector.transpose(trt[:, cs], sb[:, cs])
                    nc.scalar.activation(
                        tr[:, cs], trt[:, cs],
                        mybir.ActivationFunctionType.Relu,
                    )

            def phase_c_part(c, part):
                """pre1T jb-block {part} for steps [c*CH, (c+1)*CH):
                full-M batched GEMM from the h0T ring (half (c%2)); bias1
                folded in the PSUM->SBUF copy, which alternates between
                ACT and DVE to avoid FIFO pileups."""
                half = ring[:, (c % 2) * (CH * 256) : (c % 2 + 1) * (CH * 256)]
                rview = half.rearrange("p (t k b) -> p t k b", t=CH, b=32)
                for jb in (part,):
                    pc = psc_pool.tile([128, 512], F32, tag="psc")
                    for kb in range(KH):
                        nc.tensor.matmul(
                            pc[:, :],
                            w1_sb[:, kb * 1024 + 128 * jb : kb * 1024 + 128 * jb + 128],
                            rview[:, :, kb : kb + 1, :],
                            start=(kb == 0),
                            stop=(kb == KH - 1),
                            tile_position=(0, 0),
                            skip_group_check=True,
                        )
                    out_ap = ring2[:, :].rearrange(
                        "p (h t k b) -> p h t k b", h=2, t=CH, b=32
                    )[:, c % 2 : c % 2 + 1, :, jb : jb + 1, :]
                    in_ap = pc[:, :].rearrange("p (t b) -> p t b", b=32)
                    if jb % 2 == 0:
                        nc.scalar.activation(
                            out_ap, in_ap,
                            mybir.ActivationFunctionType.Identity,
                            bias=b1_sb[:, jb : jb + 1],
                        )
                    else:
                        nc.vector.tensor_scalar_add(
                            out_ap, in_ap, b1_sb[:, jb : jb + 1]
                        )

            hT1 = None

            def l1_step(u):
                nonlocal hT1
                off = ((u // CH) % 2) * (CH * 256) + 256 * (u % CH)
                pre1 = ring2[:, off : off + 256]
                if u == 0:
                    hT1_new = hT1_pool.tile([128, 256], BF16, tag="hT1")
                    nc.scalar.activation(
                        hT1_new[:, :], pre1, mybir.ActivationFunctionType.Relu
                    )
                    hT1 = hT1_new
                    return
                stats = [hT1[:, 32 * k : 32 * k + 32] for k in range(KH)]
                ps1 = ps1_pool.tile([128, 256], F32, tag="ps1")
                _emit_step_mms(nc, ps1, stats, w1_sb, KH, KH)
                # chain in halves: transpose (psum f32 -> sbuf f32) ->
                # +pre1T -> relu
                tr = tr1_pool.tile([128, 256], F32, tag="tr1")
                ad = ad1_pool.tile([128, 256], BF16, tag="ad1")
                hT1_new = hT1_pool.tile([128, 256], BF16, tag="hT1")
                for h in (0, 1):
                    cs = slice(128 * h, 128 * h + 128)
                    nc.vector.transpose(tr[:, cs], ps1[:, cs])
                    nc.vector.tensor_add(ad[:, cs], tr[:, cs], pre1[:, cs])
                    nc.scalar.activation(
                        hT1_new[:, cs], ad[:, cs],
                        mybir.ActivationFunctionType.Relu,
                    )
                hT1 = hT1_new

            LAG = CH + 8  # L1 lags L0: phase parts spread over 8 steps
            for u in range(T):
                l0_step(u)
                if u >= LAG:
                    l1_step(u - LAG)
                if u % CH < 8 and u >= CH:
                    # chunk (u//CH - 1)'s parts, starting one step after its
                    # last L0 step so the slot-15 chain has runway
                    phase_c_part(u // CH - 1, u % CH)
            # drain: interleave the last chunk's parts into the first 8
            # tail steps (chunk T//CH-2 consumers) so their chains get
            # matmul windows; chunk T//CH-1 consumers start at index 8.
            for i, u in enumerate(range(T - LAG, T)):
                l1_step(u)
                if i < 8:
                    phase_c_part(T // CH - 1, i)

            # ---- head: out = h1_last @ fc_w.T + fc_b ----
            hps = psh_pool.tile([BL, O], F32)
            for kb in range(KH):
                nc.tensor.matmul(
                    hps[:, :],
                    hT1[:, 32 * kb : 32 * kb + 32],
                    fcw_sb[:, kb * O : (kb + 1) * O],
                    start=(kb == 0),
                    stop=(kb == KH - 1),
                    tile_position=(0, 0),
                    skip_group_check=True,
                )
            eo = eo_pool.tile([BL, O], F32)
            nc.vector.tensor_add(eo[:, :], hps[:, :], fcb_sb[:, :])
            nc.sync.dma_start(out_d, eo[:, :])

    nc.compile()
    return nc


def _stackT(W, n_k):
    """[128, n_k*cols] bf16: [r, kb*cols + j] = W[j, 128*kb + r]."""
    cols = W.shape[0]
    WT = np.ascontiguousarray(np.asarray(W, np.float32).T)  # [in, out]
    out = np.empty((128, n_k * cols), np.float32)
    for k in range(n_k):
        out[:, k * cols : (k + 1) * cols] = WT[128 * k : 128 * (k + 1), :]
    return out.astype(ml_dtypes.bfloat16)


def _bias_il(b):
    """Interleaved bias [128, 256] f32: [32v+b', 32w+j'] = b[128w+32v+j']."""
    arr = np.asarray(b, np.float32).reshape(8, 4, 32)  # [w, v, j']
    out = np.empty((128, 256), np.float32)
    for v in range(4):
        row = np.ascontiguousarray(arr[:, v, :]).reshape(256)
        out[32 * v : 32 * v + 32, :] = row[None, :]
    return out


def _prep_core_inputs(inputs, T):
    f32 = np.float32
    w0cat = np.concatenate(
        [_stackT(np.asarray(inputs["W_ih0"], f32), KX),
         _stackT(np.asarray(inputs["W_hh0"], f32), KH)], axis=1)
    w1cat = np.concatenate(
        [_stackT(np.asarray(inputs["W_ih1"], f32), KH),
         _stackT(np.asarray(inputs["W_hh1"], f32), KH)], axis=1)
    b1 = (np.asarray(inputs["b_ih1"], f32) + np.asarray(inputs["b_hh1"], f32))
    shared = {
        "w0cat": np.ascontiguousarray(w0cat),
        "w1cat": np.ascontiguousarray(w1cat),
        "fcwT": _stackT(np.asarray(inputs["fc_w"], f32), KH),
        "bias0il": _bias_il(np.asarray(inputs["b_ih0"], f32)
                            + np.asarray(inputs["b_hh0"], f32)),
        "bias1pp": np.ascontiguousarray(b1.reshape(KH, 128).T),
        "fcb": np.tile(np.asarray(inputs["fc_b"], f32)[None, :], (BL, 1)),
    }
    x = np.asarray(inputs["input_data"], f32)  # [B, T, I]
    in_maps = []
    for c in range(N_CORES):
        xs = x[c * BL : (c + 1) * BL, :T, :]  # [BL, T, I]
        xT = np.ascontiguousarray(np.transpose(xs, (2, 1, 0))).reshape(
            I_DIM, T * BL).astype(ml_dtypes.bfloat16)
        in_maps.append(dict(shared, xTb=xT))
    return in_maps


def run(inputs, trace=False, trace_kwargs=None, T=None):
    if T is None:
        T = np.asarray(inputs["input_data"]).shape[1]
    nc = build_rnn(T)
    in_maps = _prep_core_inputs(inputs, T)
    res = run_bass_kernel_spmd(
        nc, in_maps, list(range(N_CORES)), trace=trace, **(trace_kwargs or {})
    )
    out = np.concatenate([res.results[c]["out"] for c in range(N_CORES)], axis=0)
    return out, res


def kernel(**inputs):
    return run(inputs)[0]



# revision 3
# speedup vs baseline: 1.0394x; 1.0394x over previous
"""Trainium2 Bass kernel for a 2-layer ReLU RNN (batch_first) + linear head.

Problem shapes: B=256, T=512, I=512, H=1024, O=256 (fp32).
Sharding: data-parallel over batch across 8 NeuronCores (32 rows each);
weights replicated. No collectives.

Per-core design (all matmul operands bf16, fp32 PSUM accumulate):

  L0 step t: s0 = x_t @ W_ih0.T + h0 @ W_hh0.T   (12 k-tiles, fused)
  L1 step t: s1 = pre1_t + h1 @ W_hh1.T          (8 k-tiles in-step)
  pre1 = h0 @ W_ih1.T is computed in BATCHED 16-step chunks (full-M
  stream-bound GEMMs at ~852 ns/step-equivalent instead of LDW-bound
  ~1091 ns in-step); h0T lives in a 16-slot SBUF ring and L1 lags L0
  by 16 steps. The chunk GEMM's PSUM->SBUF copy folds bias1 (per-
  partition bias in j-layout) and writes pre1T directly in hT layout.

  In-step s-matmuls run 4x column-tiled (tile_size 128x32): col-tile v
  computes the interleaved output column set {j : (j//32)%4 == v}. This
  makes the PSUM layout ps[32v+b, 32w+j'] = s[b, 128w+32v+j'], which is
  32x32-block-transpose compatible: ONE DVE StreamTranspose of
  [128, 256] yields hT[jj, 32kt+b], so the PE does no transposes.
  L0 chain: DVE add(+interleaved bias0) -> DVE transpose -> ACT relu.
  L1 chain: DVE transpose (psum) -> DVE add(+pre1T) -> ACT relu.
  Chains execute under the other layer's matmul window.

  Per-step PE budget: L0 12 rounds + L1 8 rounds (LDW-bound ~136 ns
  each) + 852 ns batched projection ~= 3.6 us/step. The 4-way col
  tiling is the exact LDW/stream balance point (tiles^2 = H*1.2GHz /
  (32 * 2.4GHz) = 16); one LDW cannot feed multiple col groups
  (verified), FWL needs 128-col loads, DoubleRow fp8 is col-tiling-
  incompatible and accuracy-breaking, so this is the toolchain floor.

  Measured: 1.88 ms HW exec (5.70 ms phase-split f32r baseline;
  2.00 ms without the batched projection), rel err 4.6e-3. Both state
  chains are emitted in 128-col halves so the first half's relu lands
  within the other layer's matmul window -- monolithic chains stall the
  PE ~300 ns/step (measured 2.13 ms with a 13.6 us monolithic chunk).
  The projection is spread ONE jb-block per step over 8 steps (L1 lag
  CH+8) so each step's engine FIFO carries at most one phase copy --
  coarser 2-per-step spreading measured +50 us of FIFO-coupling stalls.
  Residual PE idle is ~77 us, of which ~50 us is pipeline fill/drain
  at the L0/L1 lag boundaries (one ~1 us chain-latency stall per solo
  step); steady state is ~98.5% PE-busy.

kernel(**inputs) takes the FULL unsharded inputs (keys as in the
reference setup_inputs) and returns the FULL [256, 256] output.
"""

import ml_dtypes
import numpy as np

import concourse.bass as bass
import concourse.tile as tile
import concourse.mybir as mybir
from concourse import bacc
from concourse.bass_utils import run_bass_kernel_spmd

F32 = mybir.dt.float32
BF16 = mybir.dt.bfloat16

B_FULL, T_FULL, I_DIM, H, O = 256, 512, 512, 1024, 256
N_CORES = 8
BL = B_FULL // N_CORES  # 32 batch rows per core
KX = I_DIM // 128       # 4 k-tiles of the input dim
KH = H // 128           # 8 k-tiles of the hidden dim
CH = 16                 # phase-C chunk length (steps); L1 lag


def _emit_step_mms(nc, ps, stat_tiles, w_sb, kb0, n_k):
    """One recurrence step's s-matmuls, 4x column-tiled.

    ps [128,256] f32 psum; stat_tiles: n_k stationary APs [128,32] bf16;
    w_sb k-block (kb0+i) columns [(kb0+i)*1024 + j] hold W[j, 128*i + r].
    Col-tile v streams columns {j : (j//32)%4 == v} via a strided AP.
    """
    for i in range(n_k):
        blk = w_sb[:, (kb0 + i) * 1024 : (kb0 + i + 1) * 1024].rearrange(
            "p (w f j) -> p w f j", f=4, j=32
        )
        for v in range(4):
            nc.tensor.matmul(
                ps[32 * v : 32 * v + 32, :],
                stat_tiles[i],
                blk[:, :, v : v + 1, :],
                start=(i == 0),
                stop=(i == n_k - 1),
                tile_position=(0, 32 * v),
                skip_group_check=True,
            )


def build_rnn(T):
    assert T % CH == 0
    nc = bacc.Bacc("TRN2", target_bir_lowering=False, debug=False)

    xTb_d = nc.dram_tensor("xTb", [I_DIM, T * BL], BF16, kind="ExternalInput").ap()
    w0_d = nc.dram_tensor("w0cat", [128, (KX + KH) * H], BF16, kind="ExternalInput").ap()
    w1_d = nc.dram_tensor("w1cat", [128, (KH + KH) * H], BF16, kind="ExternalInput").ap()
    fcw_d = nc.dram_tensor("fcwT", [128, KH * O], BF16, kind="ExternalInput").ap()
    b0_d = nc.dram_tensor("bias0il", [128, 256], F32, kind="ExternalInput").ap()
    b1_d = nc.dram_tensor("bias1pp", [128, KH], F32, kind="ExternalInput").ap()
    fcb_d = nc.dram_tensor("fcb", [BL, O], F32, kind="ExternalInput").ap()
    out_d = nc.dram_tensor("out", [BL, O], F32, kind="ExternalOutput").ap()

    PREF = 4  # xt DMA prefetch depth (steps ahead)

    with tile.TileContext(nc) as tc:
        with (
            tc.tile_pool(name="wpool", bufs=1) as wpool,
            tc.tile_pool(name="cpool", bufs=1) as cpool,
            tc.tile_pool(name="xt", bufs=PREF + 2) as xt_pool,
            tc.tile_pool(name="hT1", bufs=3) as hT1_pool,
            tc.tile_pool(name="sb0", bufs=2) as sb0_pool,
            tc.tile_pool(name="tr1", bufs=2) as tr1_pool,
            tc.tile_pool(name="ad1", bufs=2) as ad1_pool,
            tc.tile_pool(name="ps0", bufs=2, space="PSUM") as ps0_pool,
            tc.tile_pool(name="ps1", bufs=2, space="PSUM") as ps1_pool,
            tc.tile_pool(name="psc", bufs=2, space="PSUM") as psc_pool,
            tc.tile_pool(name="psh", bufs=1, space="PSUM") as psh_pool,
            tc.tile_pool(name="eout", bufs=1) as eo_pool,
        ):
            w0_sb = wpool.tile([128, (KX + KH) * H], BF16)
            w1_sb = wpool.tile([128, (KH + KH) * H], BF16)
            fcw_sb = wpool.tile([128, KH * O], BF16)
            b0_sb = cpool.tile([128, 256], F32)
            b1_sb = cpool.tile([128, KH], F32)
            fcb_sb = cpool.tile([BL, O], F32)
            # h0T ring: 2*CH slots of [128, 256]; slot u%(2CH) = step u's h0T
            ring = cpool.tile([128, 2 * CH * 256], BF16)
            # pre1T double ring: 2 chunks x 16 slots of [128, 256]
            ring2 = cpool.tile([128, 2 * CH * 256], BF16)
            # split big weight DMAs so step 0 only gates on the x-proj
            # blocks and queues stay parallel
            nc.sync.dma_start(w0_sb[:, : KX * H], w0_d[:, : KX * H])
            nc.sync.dma_start(w0_sb[:, KX * H : (KX + 4) * H],
                              w0_d[:, KX * H : (KX + 4) * H])
            nc.sync.dma_start(w0_sb[:, (KX + 4) * H :], w0_d[:, (KX + 4) * H :])
            nc.sync.dma_start(w1_sb[:, : KH * H], w1_d[:, : KH * H])
            nc.sync.dma_start(w1_sb[:, KH * H :], w1_d[:, KH * H :])
            nc.sync.dma_start(fcw_sb[:], fcw_d)
            nc.sync.dma_start(b0_sb[:], b0_d)
            nc.sync.dma_start(b1_sb[:], b1_d)
            nc.sync.dma_start(fcb_sb[:], fcb_d)

            xT_view = xTb_d.rearrange("(ki p) n -> p ki n", p=128)

            def emit_xt_dma(t):
                xt = xt_pool.tile([128, KX * BL], BF16, tag="xt")
                nc.sync.dma_start(
                    xt[:, :].rearrange("p (ki b) -> p ki b", ki=KX),
                    xT_view[:, :, t * BL : (t + 1) * BL],
                )
                return xt

            xt_tiles = {}
            for t in range(min(T, PREF)):
                xt_tiles[t] = emit_xt_dma(t)

            def l0_step(u):
                if u + PREF < T:
                    xt_tiles[u + PREF] = emit_xt_dma(u + PREF)
                xt = xt_tiles.pop(u)
                stats = [xt[:, 32 * k : 32 * k + 32] for k in range(KX)]
                n_k = KX
                if u > 0:
                    s_prev = 256 * ((u - 1) % (2 * CH))
                    prev = ring[:, s_prev : s_prev + 256]
                    stats += [prev[:, 32 * k : 32 * k + 32] for k in range(KH)]
                    n_k += KH
                ps0 = ps0_pool.tile([128, 256], F32, tag="ps0")
                _emit_step_mms(nc, ps0, stats, w0_sb, 0, n_k)
                # chain in 128-col halves (pipelined: half A's relu lands
                # before the next step's first hT matmul needs it):
                # +bias0 (interleaved) -> 32x32 transpose -> relu
                tr = ring[:, 256 * (u % (2 * CH)) : 256 * (u % (2 * CH)) + 256]
                sb = sb0_pool.tile([128, 256], BF16, tag="sb0")
                trt = sb0_pool.tile([128, 256], BF16, tag="tr0")
                for h in (0, 1):
                    cs = slice(128 * h, 128 * h + 128)
                    nc.vector.tensor_add(sb[:, cs], ps0[:, cs], b0_sb[:, cs])
                    nc.vector.transpose(trt[:, cs], sb[:, cs])
                    nc.scalar.activation(
                        tr[:, cs], trt[:, cs],
                        mybir.ActivationFunctionType.Relu,
                    )

            def phase_c_part(c, jb, hc, g):
                """pre1T jb-block {jb} for the hc-th half (8 steps) of chunk
                c: batched GEMM from the h0T ring (half (c%2)); bias1 folded
                in the PSUM->SBUF copy, which alternates between ACT and DVE
                by global part parity to balance engine load. One such
                quarter-part runs on EVERY step so the phase matmuls cover
                the state-chain latency uniformly (spiky 8-of-16 spreading
                measured ~540 ns/step of chain stalls on the uncovered
                steps)."""
                half = ring[:, (c % 2) * (CH * 256) : (c % 2 + 1) * (CH * 256)]
                rview = half.rearrange("p (t k b) -> p t k b", t=CH, b=32)
                pc = psc_pool.tile([128, 256], F32, tag="psc")
                for kb in range(KH):
                    nc.tensor.matmul(
                        pc[:, :],
                        w1_sb[:, kb * 1024 + 128 * jb : kb * 1024 + 128 * jb + 128],
                        rview[:, 8 * hc : 8 * hc + 8, kb : kb + 1, :],
                        start=(kb == 0),
                        stop=(kb == KH - 1),
                        tile_position=(0, 0),
                        skip_group_check=True,
                    )
                out_ap = ring2[:, :].rearrange(
                    "p (h t k b) -> p h t k b", h=2, t=CH, b=32
                )[:, c % 2 : c % 2 + 1, 8 * hc : 8 * hc + 8, jb : jb + 1, :]
                in_ap = pc[:, :].rearrange("p (t b) -> p t b", b=32)
                if g % 2 == 0:
                    nc.scalar.activation(
                        out_ap, in_ap,
                        mybir.ActivationFunctionType.Identity,
                        bias=b1_sb[:, jb : jb + 1],
                    )
                else:
                    nc.vector.tensor_scalar_add(
                        out_ap, in_ap, b1_sb[:, jb : jb + 1]
                    )

            hT1 = None

            def l1_step(u):
                nonlocal hT1
                off = ((u // CH) % 2) * (CH * 256) + 256 * (u % CH)
                pre1 = ring2[:, off : off + 256]
                if u == 0:
                    hT1_new = hT1_pool.tile([128, 256], BF16, tag="hT1")
                    nc.scalar.activation(
                        hT1_new[:, :], pre1, mybir.ActivationFunctionType.Relu
                    )
                    hT1 = hT1_new
                    return
                stats = [hT1[:, 32 * k : 32 * k + 32] for k in range(KH)]
                ps1 = ps1_pool.tile([128, 256], F32, tag="ps1")
                _emit_step_mms(nc, ps1, stats, w1_sb, KH, KH)
                # chain in halves: transpose (psum f32 -> sbuf f32) ->
                # +pre1T -> relu
                tr = tr1_pool.tile([128, 256], F32, tag="tr1")
                ad = ad1_pool.tile([128, 256], BF16, tag="ad1")
                hT1_new = hT1_pool.tile([128, 256], BF16, tag="hT1")
                for h in (0, 1):
                    cs = slice(128 * h, 128 * h + 128)
                    nc.vector.transpose(tr[:, cs], ps1[:, cs])
                    nc.vector.tensor_add(ad[:, cs], tr[:, cs], pre1[:, cs])
                    nc.scalar.activation(
                        hT1_new[:, cs], ad[:, cs],
                        mybir.ActivationFunctionType.Relu,
                    )
                hT1 = hT1_new

            LAG = CH + 8  # L1 lags L0 by 24 steps
            # Quarter-part schedule: part (c, jb, hc) needs h0T of steps
            # c*CH + 8*hc .. +7, available after step c*CH + 8*hc + 7; it
            # runs at step u = c*CH + 8 + 8*hc + jb.  All 16 parts of
            # chunk c are done by step c*CH + 23, exactly in time for the
            # first chunk-c L1 step at u = c*CH + LAG.
            def part_for_step(u):
                g = u - 8
                c, r = g // CH, g % CH
                jb, hc = (r, 0) if r < 8 else (r - 8, 1)
                return c, jb, hc, g

            for u in range(T):
                l0_step(u)
                if u >= LAG:
                    l1_step(u - LAG)
                if u >= 8:
                    phase_c_part(*part_for_step(u))
            # drain: the last chunk's hc=1 parts interleave with the first
            # 8 tail L1 steps so their chains get matmul windows.
            for i, u in enumerate(range(T - LAG, T)):
                l1_step(u)
                if i < 8:
                    phase_c_part(T // CH - 1, i, 1, i)

            # ---- head: out = h1_last @ fc_w.T + fc_b ----
            hps = psh_pool.tile([BL, O], F32)
            for kb in range(KH):
                nc.tensor.matmul(
                    hps[:, :],
                    hT1[:, 32 * kb : 32 * kb + 32],
                    fcw_sb[:, kb * O : (kb + 1) * O],
                    start=(kb == 0),
                    stop=(kb == KH - 1),
                    tile_position=(0, 0),
                    skip_group_check=True,
                )
            eo = eo_pool.tile([BL, O], F32)
            nc.vector.tensor_add(eo[:, :], hps[:, :], fcb_sb[:, :])
            nc.sync.dma_start(out_d, eo[:, :])

    nc.compile()
    return nc


def _stackT(W, n_k):
    """[128, n_k*cols] bf16: [r, kb*cols + j] = W[j, 128*kb + r]."""
    cols = W.shape[0]
    WT = np.ascontiguousarray(np.asarray(W, np.float32).T)  # [in, out]
    out = np.empty((128, n_k * cols), np.float32)
    for k in range(n_k):
        out[:, k * cols : (k + 1) * cols] = WT[128 * k : 128 * (k + 1), :]
    return out.astype(ml_dtypes.bfloat16)


def _bias_il(b):
    """Interleaved bias [128, 256] f32: [32v+b', 32w+j'] = b[128w+32v+j']."""
    arr = np.asarray(b, np.float32).reshape(8, 4, 32)  # [w, v, j']
    out = np.empty((128, 256), np.float32)
    for v in range(4):
        row = np.ascontiguousarray(arr[:, v, :]).reshape(256)
        out[32 * v : 32 * v + 32, :] = row[None, :]
    return out


def _prep_core_inputs(inputs, T):
    f32 = np.float32
    w0cat = np.concatenate(
        [_stackT(np.asarray(inputs["W_ih0"], f32), KX),
         _stackT(np.asarray(inputs["W_hh0"], f32), KH)], axis=1)
    w1cat = np.concatenate(
        [_stackT(np.asarray(inputs["W_ih1"], f32), KH),
         _stackT(np.asarray(inputs["W_hh1"], f32), KH)], axis=1)
    b1 = (np.asarray(inputs["b_ih1"], f32) + np.asarray(inputs["b_hh1"], f32))
    shared = {
        "w0cat": np.ascontiguousarray(w0cat),
        "w1cat": np.ascontiguousarray(w1cat),
        "fcwT": _stackT(np.asarray(inputs["fc_w"], f32), KH),
        "bias0il": _bias_il(np.asarray(inputs["b_ih0"], f32)
                            + np.asarray(inputs["b_hh0"], f32)),
        "bias1pp": np.ascontiguousarray(b1.reshape(KH, 128).T),
        "fcb": np.tile(np.asarray(inputs["fc_b"], f32)[None, :], (BL, 1)),
    }
    x = np.asarray(inputs["input_data"], f32)  # [B, T, I]
    in_maps = []
    for c in range(N_CORES):
        xs = x[c * BL : (c + 1) * BL, :T, :]  # [BL, T, I]
        xT = np.ascontiguousarray(np.transpose(xs, (2, 1, 0))).reshape(
            I_DIM, T * BL).astype(ml_dtypes.bfloat16)
        in_maps.append(dict(shared, xTb=xT))
    return in_maps


def run(inputs, trace=False, trace_kwargs=None, T=None):
    if T is None:
        T = np.asarray(inputs["input_data"]).shape[1]
    nc = build_rnn(T)
    in_maps = _prep_core_inputs(inputs, T)
    res = run_bass_kernel_spmd(
        nc, in_maps, list(range(N_CORES)), trace=trace, **(trace_kwargs or {})
    )
    out = np.concatenate([res.results[c]["out"] for c in range(N_CORES)], axis=0)
    return out, res


def kernel(**inputs):
    return run(inputs)[0]

